# revision 7
# baseline (speedup 1.0000x reference)
"""Trainium2 Bass kernel for nn_AttentionBlock (B=8, N=2048, D=E=512).

Sharding: data-parallel over batch — each of the 8 NeuronCores computes one
batch element end-to-end (projection weights replicated on every core).
Pure SPMD, no collectives.

Per-core program (one batch element b):
  inputs : xT  [512,2048]  = x[b].T        (host pre-transposes)
           Wq/Wk/Wv [512,512], bqs (= bq/sqrt(E)) [512], bk [512], bv [512]
  output : oT  [512,2048]  = out[b].T      (host transposes back)

Math/layout on device (all matmuls in float32r — fp32 data, single-pass
reduced-precision PE mode, 4x faster than fp32 matmul; measured end-to-end
max relative error vs the fp32 reference is ~6e-5):

  QT = relu((Wq.T @ x.T)*s + bq*s)  [E,N] e-on-partitions (ACT fuses scale+bias)
  KT = relu(Wk.T @ x.T + bk)        [E,N]
  V  = relu(x @ Wv + bv)            [N,E] natural (bias added via a K=1
                                     matmul with a ones row — bias is along
                                     the free axis there)
  S^T tile [128k,512q] = KT_tile.T @ QT_chunk    (contracts e in PSUM)
  P^T = exp(S^T)     (scores lie in ~[1.5, 7] -> no max subtraction needed)
  O^T[e,q]  += V_tile.T @ P^T                    (contracts k in PSUM)
  sums[1,q] += ones.T  @ P^T                     (partition-axis reduction)
  out = O^T * broadcast(1/sums)   (1/sums broadcast across partitions via a
                                   tiny DRAM round-trip DMA + DVE multiply)

Weight-stationary orderings keep the PE's stationary operand live across 4
consecutive matmuls where the PSUM budget allows (phase 1); phase 2 keeps
the v2 fused per-kt schedule which measured fastest on hardware.
"""

import sys

if "/opt/trn_rl_repo" not in sys.path:
    sys.path.insert(0, "/opt/trn_rl_repo")

from contextlib import ExitStack

import numpy as np

import concourse.bacc as bacc
import concourse.tile as tile
from concourse import mybir
from concourse.bass_utils import run_bass_kernel_spmd

F32 = mybir.dt.float32
F32R = mybir.dt.float32r
RELU = mybir.ActivationFunctionType.Relu
EXP = mybir.ActivationFunctionType.Exp

B = 8
N = 2048
D = 512
E = 512
P = 128
NT = N // P
DT = D // P
ET = E // P
QCW = 512       # q-chunk width = one fp32 PSUM bank
QC = N // QCW
SCALE = 1.0 / float(np.sqrt(E))


def _build_nc(v_bias: bool = True):
    nc = bacc.Bacc("TRN2", num_devices=1)

    xT = nc.dram_tensor("xT", [D, N], F32, kind="ExternalInput").ap()
    wq = nc.dram_tensor("Wq", [D, E], F32, kind="ExternalInput").ap()
    wk = nc.dram_tensor("Wk", [D, E], F32, kind="ExternalInput").ap()
    wv = nc.dram_tensor("Wv", [D, E], F32, kind="ExternalInput").ap()
    bqs = nc.dram_tensor("bqs", [E], F32, kind="ExternalInput").ap()
    bk = nc.dram_tensor("bk", [E], F32, kind="ExternalInput").ap()
    bv = nc.dram_tensor("bv", [E], F32, kind="ExternalInput").ap()
    oT = nc.dram_tensor("oT", [E, N], F32, kind="ExternalOutput").ap()

    with tile.TileContext(nc) as tc:
        with ExitStack() as ctx:
            sing = ctx.enter_context(tc.tile_pool(name="singles", bufs=1))
            data = ctx.enter_context(tc.tile_pool(name="data", bufs=1))

            ones_dram_c = nc.inline_tensor(np.ones((P, 1), np.float32), name="ones_c")
            ones_dram_r = nc.inline_tensor(np.ones((1, P), np.float32), name="ones_r")
            ones_col = sing.tile([P, 1], F32R)    # lhsT for partition sums
            ones_row = sing.tile([1, P], F32R)    # lhsT for V bias matmul
            nc.sync.dma_start(out=ones_col, in_=ones_dram_c.ap().bitcast(F32R))
            nc.sync.dma_start(out=ones_row, in_=ones_dram_r.ap().bitcast(F32R))

            xt = [data.tile([P, N], F32R, name=f"xt{t}", tag=f"xt{t}") for t in range(DT)]
            wqt = [data.tile([P, E], F32R, name=f"wqt{t}", tag=f"wqt{t}") for t in range(DT)]
            wkt = [data.tile([P, E], F32R, name=f"wkt{t}", tag=f"wkt{t}") for t in range(DT)]
            wvt = [data.tile([P, E], F32R, name=f"wvt{t}", tag=f"wvt{t}") for t in range(DT)]
            qt_sb = data.tile([P, ET, N], F32R, tag="qt")
            kt_sb = data.tile([P, ET, N], F32R, tag="kt")
            v_sb = data.tile([P, NT, E], F32R, tag="v")
            bqs_sb = data.tile([P, ET], F32, tag="bqs")
            bk_sb = data.tile([P, ET], F32, tag="bk")
            bv_sb = data.tile([1, E], F32R, tag="bv")

            xTr = xT.rearrange("(t p) n -> t p n", p=P).bitcast(F32R)
            wqr = wq.rearrange("(t p) e -> t p e", p=P).bitcast(F32R)
            wkr = wk.rearrange("(t p) e -> t p e", p=P).bitcast(F32R)
            wvr = wv.rearrange("(t p) e -> t p e", p=P).bitcast(F32R)
            # order: what phase 1's first PSUM group needs goes first (all
            # four wq d-tiles + the first half of every xt d-tile), so the
            # PE can start ~6us after DMA kick-off instead of waiting for
            # the full 7MB input load
            H = N // 2
            for t in range(DT):
                nc.sync.dma_start(out=wqt[t], in_=wqr[t])
                nc.sync.dma_start(out=xt[t][:, 0:H], in_=xTr[t][:, 0:H])
            nc.sync.dma_start(out=bqs_sb, in_=bqs.rearrange("(t p) -> p t", p=P))
            for t in range(DT):
                nc.sync.dma_start(out=xt[t][:, H:N], in_=xTr[t][:, H:N])
            nc.sync.dma_start(out=bk_sb, in_=bk.rearrange("(t p) -> p t", p=P))
            nc.sync.dma_start(out=bv_sb, in_=bv.unsqueeze(0).bitcast(F32R))
            for t in range(DT):
                nc.sync.dma_start(out=wkt[t], in_=wkr[t])
            for t in range(DT):
                nc.sync.dma_start(out=wvt[t], in_=wvr[t])

            # ---------------- phase 1: Q/K/V projections ----------------
            with tc.tile_pool(name="psum1", bufs=4, space="PSUM") as psum1:
                for wt, b_sb, dst, scl in (
                    (wqt, bqs_sb, qt_sb, SCALE),
                    (wkt, bk_sb, kt_sb, 1.0),
                ):
                    for et in range(ET):
                        for qc in range(QC):
                            ps = psum1.tile([P, QCW], F32, tag="ps1")
                            for dt in range(DT):
                                nc.tensor.matmul(
                                    ps,
                                    lhsT=wt[dt][:, et * P:(et + 1) * P],
                                    rhs=xt[dt][:, qc * QCW:(qc + 1) * QCW],
                                    start=(dt == 0),
                                    stop=(dt == DT - 1),
                                )
                            nc.scalar.activation(
                                out=dst[:, et, qc * QCW:(qc + 1) * QCW],
                                in_=ps,
                                func=RELU,
                                bias=b_sb[:, et:et + 1],
                                scale=scl,
                            )
                for nt in range(NT):
                    ps = psum1.tile([P, E], F32, tag="ps1")
                    if v_bias:
                        nc.tensor.matmul(
                            ps, lhsT=ones_row, rhs=bv_sb, start=True, stop=False
                        )
                    for dt in range(DT):
                        nc.tensor.matmul(
                            ps,
                            lhsT=xt[dt][:, nt * P:(nt + 1) * P],
                            rhs=wvt[dt],
                            start=(dt == 0 and not v_bias),
                            stop=(dt == DT - 1),
                        )
                    nc.scalar.activation(out=v_sb[:, nt, :], in_=ps, func=RELU)

            # ---------------- phase 2: attention ----------------
            with (
                tc.tile_pool(name="po", bufs=5, space="PSUM") as po_pool,
                tc.tile_pool(name="psS", bufs=3, space="PSUM") as ps_pool,
                tc.tile_pool(name="rdram", bufs=2, space="DRAM") as rd_pool,
                tc.tile_pool(name="pt", bufs=3) as pt_pool,
                tc.tile_pool(name="otp", bufs=4) as ot_pool,
                tc.tile_pool(name="small", bufs=4) as small_pool,
            ):
                for qc in range(QC):
                    po = [
                        po_pool.tile([P, QCW], F32, name=f"po{e}", tag="po")
                        for e in range(ET)
                    ]
                    posum = po_pool.tile([1, QCW], F32, tag="po")
                    for kt in range(NT):
                        ps = ps_pool.tile([P, QCW], F32, tag="psS")
                        for et in range(ET):
                            nc.tensor.matmul(
                                ps,
                                lhsT=kt_sb[:, et, kt * P:(kt + 1) * P],
                                rhs=qt_sb[:, et, qc * QCW:(qc + 1) * QCW],
                                start=(et == 0),
                                stop=(et == ET - 1),
                            )
                        pt = pt_pool.tile([P, QCW], F32R, tag="pt")
                        nc.scalar.activation(out=pt, in_=ps, func=EXP)
                        nc.tensor.matmul(
                            posum,
                            lhsT=ones_col,
                            rhs=pt,
                            start=(kt == 0),
                            stop=(kt == NT - 1),
                        )
                        for et in range(ET):
                            nc.tensor.matmul(
                                po[et],
                                lhsT=v_sb[:, kt, et * P:(et + 1) * P],
                                rhs=pt,
                                start=(kt == 0),
                                stop=(kt == NT - 1),
                            )
                    rinv = small_pool.tile([1, QCW], F32, tag="rinv")
                    nc.vector.reciprocal(out=rinv, in_=posum)
                    rd = rd_pool.tile([1, QCW], F32, tag="rd")
                    nc.sync.dma_start(out=rd, in_=rinv)
                    rb = small_pool.tile([P, QCW], F32, tag="rb")
                    nc.gpsimd.dma_start(out=rb, in_=rd.to_broadcast([P, QCW]))
                    for et in range(ET):
                        ot = ot_pool.tile([P, QCW], F32, name=f"ot{et}", tag="ot")
                        nc.vector.tensor_mul(ot, po[et], rb)
                        nc.sync.dma_start(
                            out=oT[et * P:(et + 1) * P, qc * QCW:(qc + 1) * QCW],
                            in_=ot,
                        )

    nc.compile()
    return nc


_NC_CACHE = {}


def kernel(**inputs) -> np.ndarray:
    x = np.asarray(inputs["x"], dtype=np.float32)
    Wq = np.ascontiguousarray(inputs["Wq"], dtype=np.float32)
    Wk = np.ascontiguousarray(inputs["Wk"], dtype=np.float32)
    Wv = np.ascontiguousarray(inputs["Wv"], dtype=np.float32)
    bq = np.asarray(inputs["bq"], dtype=np.float32)
    bk = np.ascontiguousarray(inputs["bk"], dtype=np.float32)
    bv = np.ascontiguousarray(inputs["bv"], dtype=np.float32)

    # relu(x@Wv + 0) == relu(x@Wv): skip the 16 V-bias matmuls when bv is 0
    # (bq/bk ride the activation bias for free either way)
    v_bias = bool(np.any(bv))
    if v_bias not in _NC_CACHE:
        _NC_CACHE[v_bias] = _build_nc(v_bias=v_bias)
    nc = _NC_CACHE[v_bias]

    bqs = np.ascontiguousarray(bq * np.float32(SCALE))
    in_maps = [
        {
            "xT": np.ascontiguousarray(x[c].T),
            "Wq": Wq,
            "Wk": Wk,
            "Wv": Wv,
            "bqs": bqs,
            "bk": bk,
            "bv": bv,
        }
        for c in range(B)
    ]

    res = run_bass_kernel_spmd(nc, in_maps, core_ids=list(range(B)))
    out = np.stack(
        [np.ascontiguousarray(res.results[c]["oT"].T) for c in range(B)]
    )
    return out.astype(np.float32)
